# revision 20
# baseline (speedup 1.0000x reference)
"""Multi-head attention (B=4, S=2048, D=512, H=8) on 8 trn2 NeuronCores.

Sharding: batch*head-half per core (core c -> batch c//2, heads (c%2)*4..+4).
Each core computes its 4 heads' Q/K/V projections, scores^T, softmax (exp on
ACT with fused denominator via a ones-column in V), AV, the attention-prob
output (PE-transposed back to [s_q, s_k] and normalized during evacuation),
and its partial contribution to the final output projection. The host sums
the two per-batch partials, adds wo_b, and stitches attn shards.

Matmuls run as float32r (fp32 bits, reduced-precision PE fast path, fp32
accumulate). Output attn is exact-layout fp32.
"""

import json

import numpy as np

import concourse.bass as bass
import concourse.mybir as mybir
import concourse.tile as tile
from concourse.bass_utils import run_bass_kernel_spmd
from concourse.masks import make_identity
from concourse.vector_clock import ScopedClock, VectorClock

B, S, D, H = 4, 2048, 512, 8
DK = D // H          # 64
HPC = 4              # heads per core
P = 128
NKC = S // P         # 16 k-chunks
NG = 4               # s_q groups of 512
GS = S // NG         # 512
F32 = mybir.dt.float32
F32R = mybir.dt.float32r
AF = mybir.ActivationFunctionType
SCALE = 1.0 / np.sqrt(np.float32(DK))

# ---------------------------------------------------------------------------
# Workarounds for walrus codegen limits in this container:
#  - sequencer/CTRL-class instructions fail codegen with >1 semaphore wait
#  - Matmult waits are moved to the generated Ldweights, which supports 1
# Fix at the BIR-JSON level: move excess waits onto preceding single-wait
# NoOps on the same engine. Also replace Tile's exit drain (multi-wait) with
# per-proc single-wait drains and skip end-of-kernel sem recycling (each run
# loads a fresh NEFF, so semaphores start at zero anyway).
# ---------------------------------------------------------------------------

def _fixup_bir(d):
    for fn in d.get("functions", []):
        for blk in fn.get("blocks", []):
            out = []
            for inst in blk.get("instructions", []):
                si = inst.get("sync_info")
                ow = (si or {}).get("on_wait") or []
                limit = 1
                if len(ow) > limit:
                    keep = ow[len(ow) - limit:]
                    for k, w in enumerate(ow[: len(ow) - limit]):
                        out.append({
                            "debug": inst.get("debug", 0),
                            "engine": inst["engine"],
                            "ins": [], "outs": [],
                            "name": f"{inst['name']}-sw{k}",
                            "opcode": "NoOp",
                            "sync_info": {"on_update": [], "on_wait": [w]},
                        })
                    si["on_wait"] = keep
                out.append(inst)
            blk["instructions"] = out
    return d


def _drain_and_barrier(self, tick_clock, wait_clock):
    gc = list(tick_clock.global_clock)
    for i, v in enumerate(gc):
        if v:
            single = [0] * len(gc)
            single[i] = v
            d = self.nc.sync.drain()
            wait_clock.add_sem_waits(d.ins, ScopedClock({None: VectorClock(single)}))
    self.nc.all_engine_barrier()
    popped = self.nc._tile_sem_poison_stack.pop()
    assert popped is self._sem_poison
    self.nc.all_engine_barrier()


_installed = False


def _install_fixups():
    global _installed
    if _installed:
        return
    _installed = True
    tile.TileContext._drain_and_barrier = _drain_and_barrier
    orig = bass.Bass.to_json_bytes

    def to_json_bytes(self):
        return json.dumps(_fixup_bir(json.loads(orig(self)))).encode()

    bass.Bass.to_json_bytes = to_json_bytes

    # If BASS_TRACE is set but this image lacks antenv.axon_hooks,
    # run_bass_kernel_spmd would crash importing it; provide a no-op shim
    # (trace is then skipped gracefully). A real hook installed by a test
    # harness beforehand is left untouched.
    import sys
    import types
    try:
        import antenv
        import antenv.axon_hooks  # noqa: F401
    except ImportError:
        try:
            mod = types.ModuleType("antenv.axon_hooks")
            mod._hook = None
            mod.set_axon_ntff_profile_hook = lambda h: setattr(mod, "_hook", h)
            mod.get_axon_ntff_profile_hook = lambda: mod._hook
            sys.modules["antenv.axon_hooks"] = mod
            antenv.axon_hooks = mod
        except Exception:
            pass


# ---------------------------------------------------------------------------
# Kernel build
# ---------------------------------------------------------------------------

def _build():
    _install_fixups()
    nc = bass.Bass()
    xq = nc.dram_tensor("xq", [S, D], F32R, kind="ExternalInput")
    xk = nc.dram_tensor("xk", [S, D], F32R, kind="ExternalInput")
    xv = nc.dram_tensor("xv", [S, D], F32R, kind="ExternalInput")
    wqT = nc.dram_tensor("wqT", [D, HPC * DK], F32R, kind="ExternalInput")
    wkT = nc.dram_tensor("wkT", [D, HPC * DK], F32R, kind="ExternalInput")
    wvT = nc.dram_tensor("wvT", [D, HPC * DK], F32R, kind="ExternalInput")
    woT = nc.dram_tensor("woT", [HPC * DK, D], F32R, kind="ExternalInput")
    bq = nc.dram_tensor("bq", [HPC * DK], F32, kind="ExternalInput")
    bk = nc.dram_tensor("bk", [HPC * DK], F32, kind="ExternalInput")
    bv = nc.dram_tensor("bv", [HPC * DK], F32, kind="ExternalInput")
    attn_o = nc.dram_tensor("attn", [HPC, S, S], F32, kind="ExternalOutput")
    out_o = nc.dram_tensor("outp", [S, D], F32, kind="ExternalOutput")

    with tile.TileContext(nc) as tc:
        _body(nc, tc, xq, xk, xv, wqT, wkT, wvT, woT, bq, bk, bv, attn_o, out_o)
    return nc


def _body(nc, tc, xq, xk, xv, wqT, wkT, wvT, woT, bq, bk, bv, attn_o, out_o):
    from contextlib import ExitStack
    ctx = ExitStack()
    with ctx:
        const = ctx.enter_context(tc.tile_pool(name="const", bufs=1))
        wpool = ctx.enter_context(tc.tile_pool(name="w", bufs=1))
        proj = ctx.enter_context(tc.tile_pool(name="proj", bufs=1))
        vaugp = ctx.enter_context(tc.tile_pool(name="vaug", bufs=1))
        ctxp = ctx.enter_context(tc.tile_pool(name="ctx", bufs=1))
        smallp = ctx.enter_context(tc.tile_pool(name="small", bufs=4))
        rcp = ctx.enter_context(tc.tile_pool(name="rcp", bufs=12))
        stage = ctx.enter_context(tc.tile_pool(name="stage", bufs=6))

        ps_s = ctx.enter_context(tc.tile_pool(name="ps_s", bufs=4, space="PSUM"))
        ps_av = ctx.enter_context(tc.tile_pool(name="ps_av", bufs=1, space="PSUM"))
        ps_t = ctx.enter_context(tc.tile_pool(name="ps_t", bufs=2, space="PSUM"))
        ps_m = ctx.enter_context(tc.tile_pool(name="ps_m", bufs=1, space="PSUM"))

        # identity matrices: fp32 (exact) and an f32r copy for f32r transposes
        ident = const.tile([P, P], F32)
        make_identity(nc, ident[:])
        identr = const.tile([P, P], F32R)
        nc.vector.tensor_copy(identr[:], ident[:])
        # [1.0, 0.0] pair appended to v tiles: col 64 = ones (denominator
        # accumulator), col 65 = zeros (pad so f32r matmul dims stay even)
        aug2 = const.tile([P, 2], F32)
        nc.gpsimd.memset(aug2[:, 0:1], 1.0)
        nc.gpsimd.memset(aug2[:, 1:2], 0.0)

        # weights
        w_sb = {}
        for name, t in (("q", wqT), ("k", wkT), ("v", wvT)):
            tl = [wpool.tile([P, HPC * DK], F32R, name=f"w{name}{dc}") for dc in range(4)]
            for dc in range(4):
                nc.sync.dma_start(tl[dc][:], t[dc * P:(dc + 1) * P, :])
            w_sb[name] = tl
        woT_sb = [wpool.tile([P, D], F32R, name=f"wo{j}") for j in range(2)]
        for j in range(2):
            nc.sync.dma_start(woT_sb[j][:], woT[j * P:(j + 1) * P, :])
        b_sb = {}
        for name, t in (("q", bq), ("k", bk), ("v", bv)):
            bt = const.tile([P, 2], F32, name=f"b{name}")
            nc.sync.dma_start(bt[:], t.rearrange("(j p) -> p j", p=P))
            b_sb[name] = bt

        # -------- phase 0: transpose inputs, project q/k/v ----------------
        q4T = [proj.tile([P, S], F32R, name=f"q4T{j}") for j in range(2)]
        k4T = [proj.tile([P, S], F32R, name=f"k4T{j}") for j in range(2)]
        # v_aug[p][kc]: [128 s_k, 66] (col 64 = 1.0, col 65 = 0.0) in f32r
        v_aug = [[vaugp.tile([P, DK + 2], F32R, name=f"va{p}_{kc}")
                  for kc in range(NKC)] for p in range(HPC)]

        # phase-0-only pools, closed before the exp pool opens so their SBUF
        # is reused for the large exp working set
        with tc.tile_pool(name="xt", bufs=1) as xtp, \
             tc.tile_pool(name="load", bufs=4) as loadp, \
             tc.tile_pool(name="vproj", bufs=1) as vprojp:
            v4T = [vprojp.tile([P, S], F32R, name=f"v4T{j}") for j in range(2)]
            for name, x in (("q", xq), ("k", xk), ("v", xv)):
                y4T = {"q": q4T, "k": k4T, "v": v4T}[name]
                for g in range(NG):
                    xTg = [xtp.tile([P, GS], F32R, name=f"xTg{dc}", tag=f"xT{dc}")
                           for dc in range(4)]
                    for sc in range(4):
                        s0 = g * GS + sc * P
                        xin = loadp.tile([P, D], F32R, tag="xin", name="xin")
                        nc.sync.dma_start(xin[:], x[s0:s0 + P, :])
                        for dc in range(4):
                            pt = ps_t.tile([P, P], F32R, tag="pt", name="pt0")
                            nc.tensor.transpose(pt[:], xin[:, dc * P:(dc + 1) * P], identr[:])
                            nc.vector.tensor_copy(xTg[dc][:, sc * P:(sc + 1) * P], pt[:])
                    for j in range(2):
                        pp = ps_s.tile([P, GS], F32, tag="psc", name="pp")
                        for dc in range(4):
                            nc.tensor.matmul(
                                pp[:],
                                w_sb[name][dc][:, j * P:(j + 1) * P],
                                xTg[dc][:],
                                start=(dc == 0), stop=(dc == 3),
                            )
                        nc.scalar.activation(
                            y4T[j][:, g * GS:(g + 1) * GS], pp[:],
                            AF.Identity, bias=b_sb[name][:, j:j + 1],
                        )
            # v4T [2-head dk, S] -> v_aug tiles [s_k, 66]
            for j in range(2):
                for kc in range(NKC):
                    pt = ps_t.tile([P, P], F32R, tag="pt", name="ptv")
                    nc.tensor.transpose(pt[:], v4T[j][:, kc * P:(kc + 1) * P], identr[:])
                    for jj in range(2):
                        va = v_aug[2 * j + jj][kc]
                        nc.vector.tensor_copy(va[:, 0:DK], pt[:, jj * DK:(jj + 1) * DK])
                        nc.vector.tensor_copy(va[:, DK:DK + 2], aug2[:])

        # ctxT (normalized, f32r) for the WO matmul: [j][128 (2-head dk), S]
        ctxT = [ctxp.tile([P, S], F32R, name=f"ctxT{j}") for j in range(2)]

        # -------- phase 1: per (head, s_q group), software-pipelined ------
        # The attn-output transposes + evacuation + DMA of iteration i are
        # emitted interleaved into iteration i+1's scores loop, so the PE's
        # in-order stream always has ready transpose work between scores
        # matmuls (which are paced by the exp evacuations on ACT). This
        # keeps the PE busy -> HAM stays at K=8/8.
        expp = ctx.enter_context(tc.tile_pool(name="exp", bufs=38))

        def emit_out_slot(prev, slot):
            pp_, pg_, pets, precips = prev
            sub, kq = divmod(slot, 4)
            pt = ps_t.tile([P, GS], F32R, tag="pt", name="pt1")
            for t in range(4):
                kc = kq * 4 + t
                nc.tensor.transpose(
                    pt[:, t * P:(t + 1) * P],
                    pets[kc][:, sub * P:(sub + 1) * P],
                    identr[:])
            at = stage.tile([P, GS], F32, tag="at", name="at")
            # split evacuations across ACT and DVE: the DVE queue otherwise
            # backs up the ps_t bank rotation and stalls the PE transposes
            if slot % 3 == 0:
                nc.scalar.mul(at[:], pt[:], precips[sub][:])
            else:
                nc.vector.tensor_scalar_mul(at[:], pt[:], precips[sub][:])
            r0 = pg_ * GS + sub * P
            nc.sync.dma_start(
                attn_o[pp_, r0:r0 + P, kq * GS:(kq + 1) * GS], at[:])

        prev = None
        for p in range(HPC):
            j, rh = divmod(p, 2)
            qs = q4T[j][rh * DK:(rh + 1) * DK, :]
            ks = k4T[j][rh * DK:(rh + 1) * DK, :]
            for g in range(NG):
                ets = []
                for kc in range(NKC):
                    if prev is not None:
                        emit_out_slot(prev, kc)
                    ps = ps_s.tile([P, GS], F32, tag="psc", name="ps")
                    nc.tensor.matmul(
                        ps[:], ks[:, kc * P:(kc + 1) * P], qs[:, g * GS:(g + 1) * GS],
                        start=True, stop=True,
                    )
                    et = expp.tile([P, GS], F32R, tag="exp", name="et")
                    nc.scalar.activation(et[:], ps[:], AF.Exp, scale=float(SCALE))
                    ets.append(et)
                # AV + denominator as a contiguous accumulation burst (an
                # interleaved AV measurably re-throttles the PE via
                # psum-queue depth cycling)
                pav = ps_av.tile([DK + 2, GS], F32, tag="pav", name="pav")
                for kc in range(NKC):
                    nc.tensor.matmul(
                        pav[:], v_aug[p][kc][:], ets[kc][:],
                        start=(kc == 0), stop=(kc == NKC - 1),
                    )
                # ctx normalize minis: evacuations on ACT so the PE never
                # waits on the (busy) DVE; only the reciprocal stays on DVE
                ctu = smallp.tile([DK + 2, GS], F32R, tag="ctu", name="ctu")
                nc.scalar.copy(ctu[:], pav[:])
                recips = []
                for sub in range(4):
                    p2 = ps_m.tile([P, DK + 2], F32R, tag="pm", name="p2")
                    nc.tensor.transpose(
                        p2[:], ctu[:, sub * P:(sub + 1) * P],
                        identr[:DK + 2, :DK + 2])
                    rc = rcp.tile([P, 1], F32, tag="rc", name="rc")
                    nc.vector.reciprocal(rc[:], p2[:, DK:DK + 1])
                    recips.append(rc)
                    cn = smallp.tile([P, DK], F32R, tag="cn", name="cn")
                    nc.scalar.mul(cn[:], p2[:, 0:DK], rc[:])
                    p3 = ps_m.tile([DK, P], F32R, tag="pm", name="p3")
                    nc.tensor.transpose(p3[:], cn[:], identr[:])
                    nc.scalar.copy(
                        ctxT[j][rh * DK:(rh + 1) * DK,
                                g * GS + sub * P: g * GS + (sub + 1) * P],
                        p3[:])
                prev = (p, g, ets, recips)

        # -------- phase 2: output projection + drain of last iteration ----
        slot = 0
        for g in range(NG):
            for sub in range(4):
                if prev is not None and slot < NKC:
                    emit_out_slot(prev, slot)
                slot += 1
                r0 = g * GS + sub * P
                po = ps_s.tile([P, D], F32, tag="psc", name="po")
                for j in range(2):
                    nc.tensor.matmul(
                        po[:], ctxT[j][:, r0:r0 + P], woT_sb[j][:],
                        start=(j == 0), stop=(j == 1),
                    )
                ot = stage.tile([P, D], F32, tag="at", name="ot")
                nc.vector.tensor_copy(ot[:], po[:])
                nc.sync.dma_start(out_o[r0:r0 + P, :], ot[:])


_NC = None
LAST_RESULT = None


def kernel(query, key, value, wq_w, wq_b, wk_w, wk_b, wv_w, wv_b, wo_w, wo_b):
    global _NC, LAST_RESULT
    query = np.asarray(query, np.float32)
    key = np.asarray(key, np.float32)
    value = np.asarray(value, np.float32)
    wq_w = np.asarray(wq_w, np.float32)
    wk_w = np.asarray(wk_w, np.float32)
    wv_w = np.asarray(wv_w, np.float32)
    wo_w = np.asarray(wo_w, np.float32)
    wq_b = np.asarray(wq_b, np.float32)
    wk_b = np.asarray(wk_b, np.float32)
    wv_b = np.asarray(wv_b, np.float32)
    wo_b = np.asarray(wo_b, np.float32)

    if _NC is None:
        _NC = _build()

    in_maps = []
    for c in range(8):
        b, hh = divmod(c, 2)
        rows = slice(hh * HPC * DK, (hh + 1) * HPC * DK)
        in_maps.append({
            "xq": np.ascontiguousarray(query[b]),
            "xk": np.ascontiguousarray(key[b]),
            "xv": np.ascontiguousarray(value[b]),
            "wqT": np.ascontiguousarray(wq_w[rows, :].T),
            "wkT": np.ascontiguousarray(wk_w[rows, :].T),
            "wvT": np.ascontiguousarray(wv_w[rows, :].T),
            "woT": np.ascontiguousarray(wo_w[:, rows].T),
            "bq": np.ascontiguousarray(wq_b[rows]),
            "bk": np.ascontiguousarray(wk_b[rows]),
            "bv": np.ascontiguousarray(wv_b[rows]),
        })
    try:
        res = run_bass_kernel_spmd(_NC, in_maps, core_ids=list(range(8)))
    except Exception:
        # one retry: a transiently wedged NeuronCore usually recovers on the
        # next session (NRT_EXEC_UNIT_UNRECOVERABLE seen once under load)
        res = run_bass_kernel_spmd(_NC, in_maps, core_ids=list(range(8)))
    LAST_RESULT = res

    attn = np.empty((B, H, S, S), np.float32)
    out = np.empty((B, S, D), np.float32)
    for c in range(8):
        b, hh = divmod(c, 2)
        attn[b, hh * HPC:(hh + 1) * HPC] = res.results[c]["attn"]
    for b in range(B):
        out[b] = res.results[2 * b]["outp"] + res.results[2 * b + 1]["outp"] + wo_b
    return out, attn


# revision 21
# speedup vs baseline: 1.3737x; 1.3737x over previous
"""Multi-head attention (B=4, S=2048, D=512, H=8) on 8 trn2 NeuronCores.

Sharding: batch*head-half per core (core c -> batch c//2, heads (c%2)*4..+4).
Each core computes its 4 heads' Q/K/V projections, scores^T, softmax (exp on
ACT with fused denominator via a ones-column in V), AV, the attention-prob
output (PE-transposed back to [s_q, s_k] and normalized during evacuation),
and its partial contribution to the final output projection. The host sums
the two per-batch partials, adds wo_b, and stitches attn shards.

Matmuls run as float32r (fp32 bits, reduced-precision PE fast path, fp32
accumulate). Output attn is exact-layout fp32.
"""

import json

import numpy as np

import concourse.bass as bass
import concourse.mybir as mybir
import concourse.tile as tile
from concourse.bass_utils import run_bass_kernel_spmd
from concourse.masks import make_identity
from concourse.vector_clock import ScopedClock, VectorClock

B, S, D, H = 4, 2048, 512, 8
DK = D // H          # 64
HPC = 4              # heads per core
P = 128
NKC = S // P         # 16 k-chunks
NG = 4               # s_q groups of 512
GS = S // NG         # 512
F32 = mybir.dt.float32
F32R = mybir.dt.float32r
AF = mybir.ActivationFunctionType
SCALE = 1.0 / np.sqrt(np.float32(DK))

# ---------------------------------------------------------------------------
# Workarounds for walrus codegen limits in this container:
#  - sequencer/CTRL-class instructions fail codegen with >1 semaphore wait
#  - Matmult waits are moved to the generated Ldweights, which supports 1
# Fix at the BIR-JSON level: move excess waits onto preceding single-wait
# NoOps on the same engine. Also replace Tile's exit drain (multi-wait) with
# per-proc single-wait drains and skip end-of-kernel sem recycling (each run
# loads a fresh NEFF, so semaphores start at zero anyway).
# ---------------------------------------------------------------------------

def _fixup_bir(d):
    for fn in d.get("functions", []):
        for blk in fn.get("blocks", []):
            out = []
            for inst in blk.get("instructions", []):
                si = inst.get("sync_info")
                ow = (si or {}).get("on_wait") or []
                limit = 1
                if len(ow) > limit:
                    keep = ow[len(ow) - limit:]
                    for k, w in enumerate(ow[: len(ow) - limit]):
                        out.append({
                            "debug": inst.get("debug", 0),
                            "engine": inst["engine"],
                            "ins": [], "outs": [],
                            "name": f"{inst['name']}-sw{k}",
                            "opcode": "NoOp",
                            "sync_info": {"on_update": [], "on_wait": [w]},
                        })
                    si["on_wait"] = keep
                out.append(inst)
            blk["instructions"] = out
    return d


def _drain_and_barrier(self, tick_clock, wait_clock):
    gc = list(tick_clock.global_clock)
    for i, v in enumerate(gc):
        if v:
            single = [0] * len(gc)
            single[i] = v
            d = self.nc.sync.drain()
            wait_clock.add_sem_waits(d.ins, ScopedClock({None: VectorClock(single)}))
    self.nc.all_engine_barrier()
    popped = self.nc._tile_sem_poison_stack.pop()
    assert popped is self._sem_poison
    self.nc.all_engine_barrier()


_installed = False


def _install_fixups():
    global _installed
    if _installed:
        return
    _installed = True
    tile.TileContext._drain_and_barrier = _drain_and_barrier
    orig = bass.Bass.to_json_bytes

    def to_json_bytes(self):
        return json.dumps(_fixup_bir(json.loads(orig(self)))).encode()

    bass.Bass.to_json_bytes = to_json_bytes

    # If BASS_TRACE is set but this image lacks antenv.axon_hooks,
    # run_bass_kernel_spmd would crash importing it; provide a no-op shim
    # (trace is then skipped gracefully). A real hook installed by a test
    # harness beforehand is left untouched.
    import sys
    import types
    try:
        import antenv
        import antenv.axon_hooks  # noqa: F401
    except ImportError:
        try:
            mod = types.ModuleType("antenv.axon_hooks")
            mod._hook = None
            mod.set_axon_ntff_profile_hook = lambda h: setattr(mod, "_hook", h)
            mod.get_axon_ntff_profile_hook = lambda: mod._hook
            sys.modules["antenv.axon_hooks"] = mod
            antenv.axon_hooks = mod
        except Exception:
            pass


# ---------------------------------------------------------------------------
# Kernel build
# ---------------------------------------------------------------------------

def _build():
    _install_fixups()
    nc = bass.Bass()
    xq = nc.dram_tensor("xq", [S, D], F32R, kind="ExternalInput")
    xk = nc.dram_tensor("xk", [S, D], F32R, kind="ExternalInput")
    xv = nc.dram_tensor("xv", [S, D], F32R, kind="ExternalInput")
    wqT = nc.dram_tensor("wqT", [D, HPC * DK], F32R, kind="ExternalInput")
    wkT = nc.dram_tensor("wkT", [D, HPC * DK], F32R, kind="ExternalInput")
    wvT = nc.dram_tensor("wvT", [D, HPC * DK], F32R, kind="ExternalInput")
    woT = nc.dram_tensor("woT", [HPC * DK, D], F32R, kind="ExternalInput")
    bq = nc.dram_tensor("bq", [HPC * DK], F32, kind="ExternalInput")
    bk = nc.dram_tensor("bk", [HPC * DK], F32, kind="ExternalInput")
    bv = nc.dram_tensor("bv", [HPC * DK], F32, kind="ExternalInput")
    attn_o = nc.dram_tensor("attn", [HPC, S, S], F32, kind="ExternalOutput")
    out_o = nc.dram_tensor("outp", [S, D], F32, kind="ExternalOutput")

    with tile.TileContext(nc) as tc:
        _body(nc, tc, xq, xk, xv, wqT, wkT, wvT, woT, bq, bk, bv, attn_o, out_o)
    return nc


def _body(nc, tc, xq, xk, xv, wqT, wkT, wvT, woT, bq, bk, bv, attn_o, out_o):
    from contextlib import ExitStack
    ctx = ExitStack()
    with ctx:
        const = ctx.enter_context(tc.tile_pool(name="const", bufs=1))
        wpool = ctx.enter_context(tc.tile_pool(name="w", bufs=1))
        proj = ctx.enter_context(tc.tile_pool(name="proj", bufs=1))
        vaugp = ctx.enter_context(tc.tile_pool(name="vaug", bufs=1))
        ctxp = ctx.enter_context(tc.tile_pool(name="ctx", bufs=1))
        smallp = ctx.enter_context(tc.tile_pool(name="small", bufs=4))
        rcp = ctx.enter_context(tc.tile_pool(name="rcp", bufs=12))
        stage = ctx.enter_context(tc.tile_pool(name="stage", bufs=6))

        ps_s = ctx.enter_context(tc.tile_pool(name="ps_s", bufs=4, space="PSUM"))
        ps_av = ctx.enter_context(tc.tile_pool(name="ps_av", bufs=1, space="PSUM"))
        ps_t = ctx.enter_context(tc.tile_pool(name="ps_t", bufs=2, space="PSUM"))
        ps_m = ctx.enter_context(tc.tile_pool(name="ps_m", bufs=1, space="PSUM"))

        # identity matrices: fp32 (exact) and an f32r copy for f32r transposes
        ident = const.tile([P, P], F32)
        make_identity(nc, ident[:])
        identr = const.tile([P, P], F32R)
        nc.vector.tensor_copy(identr[:], ident[:])
        # [1.0, 0.0] pair appended to v tiles: col 64 = ones (denominator
        # accumulator), col 65 = zeros (pad so f32r matmul dims stay even)
        aug2 = const.tile([P, 2], F32)
        nc.gpsimd.memset(aug2[:, 0:1], 1.0)
        nc.gpsimd.memset(aug2[:, 1:2], 0.0)

        # weights
        w_sb = {}
        for name, t in (("q", wqT), ("k", wkT), ("v", wvT)):
            tl = [wpool.tile([P, HPC * DK], F32R, name=f"w{name}{dc}") for dc in range(4)]
            for dc in range(4):
                nc.sync.dma_start(tl[dc][:], t[dc * P:(dc + 1) * P, :])
            w_sb[name] = tl
        woT_sb = [wpool.tile([P, D], F32R, name=f"wo{j}") for j in range(2)]
        for j in range(2):
            nc.sync.dma_start(woT_sb[j][:], woT[j * P:(j + 1) * P, :])
        b_sb = {}
        for name, t in (("q", bq), ("k", bk), ("v", bv)):
            bt = const.tile([P, 2], F32, name=f"b{name}")
            nc.sync.dma_start(bt[:], t.rearrange("(j p) -> p j", p=P))
            b_sb[name] = bt

        # -------- phase 0: transpose inputs, project q/k/v ----------------
        q4T = [proj.tile([P, S], F32R, name=f"q4T{j}") for j in range(2)]
        k4T = [proj.tile([P, S], F32R, name=f"k4T{j}") for j in range(2)]
        # v_aug[p][kc]: [128 s_k, 66] (col 64 = 1.0, col 65 = 0.0) in f32r
        v_aug = [[vaugp.tile([P, DK + 2], F32R, name=f"va{p}_{kc}")
                  for kc in range(NKC)] for p in range(HPC)]

        # phase-0-only pools, closed before the exp pool opens so their SBUF
        # is reused for the large exp working set
        with tc.tile_pool(name="xt", bufs=1) as xtp, \
             tc.tile_pool(name="load", bufs=4) as loadp, \
             tc.tile_pool(name="vproj", bufs=1) as vprojp:
            v4T = [vprojp.tile([P, S], F32R, name=f"v4T{j}") for j in range(2)]
            for name, x in (("q", xq), ("k", xk), ("v", xv)):
                y4T = {"q": q4T, "k": k4T, "v": v4T}[name]
                for g in range(NG):
                    xTg = [xtp.tile([P, GS], F32R, name=f"xTg{dc}", tag=f"xT{dc}")
                           for dc in range(4)]
                    for sc in range(4):
                        s0 = g * GS + sc * P
                        xin = loadp.tile([P, D], F32R, tag="xin", name="xin")
                        nc.sync.dma_start(xin[:], x[s0:s0 + P, :])
                        for dc in range(4):
                            pt = ps_t.tile([P, P], F32R, tag="pt", name="pt0")
                            nc.tensor.transpose(pt[:], xin[:, dc * P:(dc + 1) * P], identr[:])
                            nc.vector.tensor_copy(xTg[dc][:, sc * P:(sc + 1) * P], pt[:])
                    for j in range(2):
                        pp = ps_s.tile([P, GS], F32, tag="psc", name="pp")
                        for dc in range(4):
                            nc.tensor.matmul(
                                pp[:],
                                w_sb[name][dc][:, j * P:(j + 1) * P],
                                xTg[dc][:],
                                start=(dc == 0), stop=(dc == 3),
                            )
                        nc.scalar.activation(
                            y4T[j][:, g * GS:(g + 1) * GS], pp[:],
                            AF.Identity, bias=b_sb[name][:, j:j + 1],
                        )
            # v4T [2-head dk, S] -> v_aug tiles [s_k, 66]
            for j in range(2):
                for kc in range(NKC):
                    pt = ps_t.tile([P, P], F32R, tag="pt", name="ptv")
                    nc.tensor.transpose(pt[:], v4T[j][:, kc * P:(kc + 1) * P], identr[:])
                    for jj in range(2):
                        va = v_aug[2 * j + jj][kc]
                        nc.vector.tensor_copy(va[:, 0:DK], pt[:, jj * DK:(jj + 1) * DK])
                        nc.vector.tensor_copy(va[:, DK:DK + 2], aug2[:])

        # ctxT (normalized, f32r) for the WO matmul: [j][128 (2-head dk), S]
        ctxT = [ctxp.tile([P, S], F32R, name=f"ctxT{j}") for j in range(2)]

        # -------- phase 1: per (head, s_q group), software-pipelined ------
        # The attn-output transposes + evacuation + DMA of iteration i are
        # emitted interleaved into iteration i+1's scores loop, so the PE's
        # in-order stream always has ready transpose work between scores
        # matmuls (which are paced by the exp evacuations on ACT). This
        # keeps the PE busy -> HAM stays at K=8/8.
        expp = ctx.enter_context(tc.tile_pool(name="exp", bufs=38))

        def emit_out_slot(prev, slot):
            pp_, pg_, pets, precips = prev
            sub, kq = divmod(slot, 4)
            pt = ps_t.tile([P, GS], F32R, tag="pt", name="pt1")
            for t in range(4):
                kc = kq * 4 + t
                nc.tensor.transpose(
                    pt[:, t * P:(t + 1) * P],
                    pets[kc][:, sub * P:(sub + 1) * P],
                    identr[:])
            at = stage.tile([P, GS], F32, tag="at", name="at")
            nc.vector.tensor_scalar_mul(at[:], pt[:], precips[sub][:])
            r0 = pg_ * GS + sub * P
            nc.sync.dma_start(
                attn_o[pp_, r0:r0 + P, kq * GS:(kq + 1) * GS], at[:])

        prev = None
        for p in range(HPC):
            j, rh = divmod(p, 2)
            qs = q4T[j][rh * DK:(rh + 1) * DK, :]
            ks = k4T[j][rh * DK:(rh + 1) * DK, :]
            for g in range(NG):
                ets = []
                for kc in range(NKC):
                    if prev is not None:
                        emit_out_slot(prev, kc)
                    ps = ps_s.tile([P, GS], F32, tag="psc", name="ps")
                    nc.tensor.matmul(
                        ps[:], ks[:, kc * P:(kc + 1) * P], qs[:, g * GS:(g + 1) * GS],
                        start=True, stop=True,
                    )
                    et = expp.tile([P, GS], F32R, tag="exp", name="et")
                    nc.scalar.activation(et[:], ps[:], AF.Exp, scale=float(SCALE))
                    ets.append(et)
                # AV + denominator as a contiguous accumulation burst (an
                # interleaved AV measurably re-throttles the PE via
                # psum-queue depth cycling)
                pav = ps_av.tile([DK + 2, GS], F32, tag="pav", name="pav")
                for kc in range(NKC):
                    nc.tensor.matmul(
                        pav[:], v_aug[p][kc][:], ets[kc][:],
                        start=(kc == 0), stop=(kc == NKC - 1),
                    )
                # ctx normalize minis: evacuations on ACT so the PE never
                # waits on the (busy) DVE; only the reciprocal stays on DVE
                ctu = smallp.tile([DK + 2, GS], F32R, tag="ctu", name="ctu")
                nc.scalar.copy(ctu[:], pav[:])
                recips = []
                for sub in range(4):
                    p2 = ps_m.tile([P, DK + 2], F32R, tag="pm", name="p2")
                    nc.tensor.transpose(
                        p2[:], ctu[:, sub * P:(sub + 1) * P],
                        identr[:DK + 2, :DK + 2])
                    rc = rcp.tile([P, 1], F32, tag="rc", name="rc")
                    nc.vector.reciprocal(rc[:], p2[:, DK:DK + 1])
                    recips.append(rc)
                    cn = smallp.tile([P, DK], F32R, tag="cn", name="cn")
                    nc.scalar.mul(cn[:], p2[:, 0:DK], rc[:])
                    p3 = ps_m.tile([DK, P], F32R, tag="pm", name="p3")
                    nc.tensor.transpose(p3[:], cn[:], identr[:])
                    nc.scalar.copy(
                        ctxT[j][rh * DK:(rh + 1) * DK,
                                g * GS + sub * P: g * GS + (sub + 1) * P],
                        p3[:])
                prev = (p, g, ets, recips)

        # -------- phase 2: output projection + drain of last iteration ----
        slot = 0
        for g in range(NG):
            for sub in range(4):
                if prev is not None and slot < NKC:
                    emit_out_slot(prev, slot)
                slot += 1
                r0 = g * GS + sub * P
                po = ps_s.tile([P, D], F32, tag="psc", name="po")
                for j in range(2):
                    nc.tensor.matmul(
                        po[:], ctxT[j][:, r0:r0 + P], woT_sb[j][:],
                        start=(j == 0), stop=(j == 1),
                    )
                ot = stage.tile([P, D], F32, tag="at", name="ot")
                nc.vector.tensor_copy(ot[:], po[:])
                nc.sync.dma_start(out_o[r0:r0 + P, :], ot[:])


_NC = None
LAST_RESULT = None


def kernel(query, key, value, wq_w, wq_b, wk_w, wk_b, wv_w, wv_b, wo_w, wo_b):
    global _NC, LAST_RESULT
    query = np.asarray(query, np.float32)
    key = np.asarray(key, np.float32)
    value = np.asarray(value, np.float32)
    wq_w = np.asarray(wq_w, np.float32)
    wk_w = np.asarray(wk_w, np.float32)
    wv_w = np.asarray(wv_w, np.float32)
    wo_w = np.asarray(wo_w, np.float32)
    wq_b = np.asarray(wq_b, np.float32)
    wk_b = np.asarray(wk_b, np.float32)
    wv_b = np.asarray(wv_b, np.float32)
    wo_b = np.asarray(wo_b, np.float32)

    if _NC is None:
        _NC = _build()

    in_maps = []
    for c in range(8):
        b, hh = divmod(c, 2)
        rows = slice(hh * HPC * DK, (hh + 1) * HPC * DK)
        in_maps.append({
            "xq": np.ascontiguousarray(query[b]),
            "xk": np.ascontiguousarray(key[b]),
            "xv": np.ascontiguousarray(value[b]),
            "wqT": np.ascontiguousarray(wq_w[rows, :].T),
            "wkT": np.ascontiguousarray(wk_w[rows, :].T),
            "wvT": np.ascontiguousarray(wv_w[rows, :].T),
            "woT": np.ascontiguousarray(wo_w[:, rows].T),
            "bq": np.ascontiguousarray(wq_b[rows]),
            "bk": np.ascontiguousarray(wk_b[rows]),
            "bv": np.ascontiguousarray(wv_b[rows]),
        })
    try:
        res = run_bass_kernel_spmd(_NC, in_maps, core_ids=list(range(8)))
    except Exception:
        # one retry: a transiently wedged NeuronCore usually recovers on the
        # next session (NRT_EXEC_UNIT_UNRECOVERABLE seen once under load)
        res = run_bass_kernel_spmd(_NC, in_maps, core_ids=list(range(8)))
    LAST_RESULT = res

    attn = np.empty((B, H, S, S), np.float32)
    out = np.empty((B, S, D), np.float32)
    for c in range(8):
        b, hh = divmod(c, 2)
        attn[b, hh * HPC:(hh + 1) * HPC] = res.results[c]["attn"]
    for b in range(B):
        out[b] = res.results[2 * b]["outp"] + res.results[2 * b + 1]["outp"] + wo_b
    return out, attn
